# revision 27
# baseline (speedup 1.0000x reference)
"""Batched GNN neighbor aggregation on 8 NeuronCores.

out[b] = neibors[b] @ last_embs[b]  for b in 0..7  (2048x2048 @ 2048x128, f32)

Sharding: one graph per core (batch dim across the 8 cores), no cross-core
communication. The PE contracts over the partition dimension, so the
adjacency operand must sit in SBUF with the contraction index (m) on
partitions; each graph's adjacency is pre-transposed on the host during
sharding so the device streams it with fully-contiguous DMAs.

Precision scheme (TRN2's native fp32 matmul is 4 cycles/row and slower
than the HBM stream): A = bf16 hi + fp8e4m3 lo (lo scaled by 2^9),
E = bf16 hi + bf16 lo. Per k-chunk the device accumulates in f32 PSUM:
  Ah@Eh + Ah@El          (bf16, 1 cycle/row)
  Al8@(Eh * 2^-9 as fp8) (fp8 DoubleRow over k-chunk pairs, 0.5 cyc/row)
The 2^9/2^-9 scales are powers of two and cancel exactly. Measured error
vs the f32 reference: absmax-rel 4.3e-4, resid_var 1.8e-7.

Stream is ~13.8 MB/core (vs 18.4 full-f32), PE ~40 us: measured
~58-62 us per core wall in quiet windows (up to ~70 us when the shared
HBM stacks see external contention), including ~8.5 us fixed NEFF/Tile
preamble and ~6 us tail. A ~3.4us scratch-matmul pre-warm during the DMA
preamble keeps the PE HAM clock at 2.4GHz for the real matmuls. The
device computes out^T = embs^T @ neibors^T with the embedding K-chunks
stationary; the host transposes the small result back.
"""

import numpy as np
import ml_dtypes

BF16 = ml_dtypes.bfloat16
FP8 = ml_dtypes.float8_e4m3
LO_SCALE = np.float32(512.0)

B = 8
N = 2048
D = 128
KT = 128
NT = 512
NK = N // KT   # 16
NKH = NK // 2  # 8 k-chunk pairs for DoubleRow
NN = N // NT   # 4

_cached_nc = None


def _dedup_ldweights(nc, mybir):
    """Drop InstLdweights whose weight AP matches the immediately preceding
    weight load in the PE stream (matmuls here have ldweights=False, so the
    stationary operand stays in the array between identical loads)."""
    for bb in nc.m.functions[0].blocks:
        insts = bb.instructions
        last_key = None
        removed = []
        for inst in insts:
            if getattr(inst, "engine", None) != mybir.EngineType.PE:
                continue
            ty = type(inst).__name__
            if ty == "InstLdweights":
                key = repr(inst.ins[0])
                if key == last_key and not inst.has_wait():
                    removed.append(inst)
                else:
                    last_key = key
            elif ty != "InstMatmult":
                last_key = None
        if removed:
            rm = {id(i) for i in removed}
            insts[:] = [i for i in insts if id(i) not in rm]
            for i in removed:
                nc.inst_map.pop(i.name, None)


def _build_program():
    import concourse.tile as tile
    from concourse import bacc, mybir

    f32 = mybir.dt.float32
    bf16 = mybir.dt.bfloat16
    fp8 = mybir.dt.float8e4
    DR = mybir.MatmulPerfMode.DoubleRow
    nc = bacc.Bacc(
        "TRN2",
        target_bir_lowering=False,
        debug=False,
        enable_asserts=False,
        enable_partition_id=False,
    )

    a_hi = nc.dram_tensor("a_hi", [NK, KT, N], bf16, kind="ExternalInput")
    a_lo = nc.dram_tensor("a_lo", [NK, KT, N], fp8, kind="ExternalInput")
    # e2[plane, p, k, d]: 0 = Eh, 1 = El (bf16)
    e2 = nc.dram_tensor("e2", [2, KT, NK, D], bf16, kind="ExternalInput")
    # e8[j, p, i, d] = fp8(Eh * 2^-9) for k-chunk 2j+i (DoubleRow weights)
    e8 = nc.dram_tensor("e8", [NKH, KT, 2, D], fp8, kind="ExternalInput")
    out_t = nc.dram_tensor("out_t", [D, N], f32, kind="ExternalOutput")

    with tile.TileContext(nc) as tc:
        with (
            tc.tile_pool(name="econst", bufs=1) as epool,
            tc.tile_pool(name="ahi", bufs=8) as hpool,
            tc.tile_pool(name="alo", bufs=4) as lpool,
            tc.tile_pool(name="psum", bufs=1, space="PSUM") as pspool,
            tc.tile_pool(name="out", bufs=1) as opool,
        ):
            # HAM pre-warm: ~3.4us of scratch matmuls during the DMA-wait
            # preamble so the real matmuls start at 2.4GHz, not 1.2GHz.
            wu = epool.tile([KT, KT], bf16, name="wu")
            wu_ps = pspool.tile([KT, KT], f32, name="wups", tag="wups")
            nc.gpsimd.memset(wu[:], 0.0)
            for _ in range(32):
                nc.tensor.matmul(wu_ps[:], wu[:], wu[:], start=True, stop=True)

            e2_r = e2.ap().rearrange("s p k d -> p s k d")
            e_sb = epool.tile([KT, 2, NK, D], bf16)
            e8_sb = epool.tile([KT, NKH, 2, D], fp8, name="e8_sb")
            nc.sync.dma_start(e_sb[:, 0, 0], e2_r[:, 0, 0])
            nc.scalar.dma_start(e_sb[:, 0, 1:], e2_r[:, 0, 1:])
            nc.scalar.dma_start(e_sb[:, 1], e2_r[:, 1])
            nc.scalar.dma_start(e8_sb[:], e8.ap().rearrange("j p i d -> p j i d"))

            ps = [
                pspool.tile([D, NT], f32, name=f"ps{n}", tag=f"ps{n}")
                for n in range(NN)
            ]

            lo_pairs = {}
            for k in range(NK):
                hi = hpool.tile([KT, N], bf16, tag="hi")
                if k == 0:
                    for n in range(NN):
                        nc.sync.dma_start(
                            hi[:, n * NT : (n + 1) * NT],
                            a_hi.ap()[k][:, n * NT : (n + 1) * NT],
                        )
                elif k == NK - 1:
                    for n in range(NN):
                        nc.sync.dma_start(
                            hi[:, n * NT : (n + 1) * NT],
                            a_hi.ap()[k][:, n * NT : (n + 1) * NT],
                        )
                else:
                    nc.sync.dma_start(hi[:], a_hi.ap()[k])
                if k % 2 == 0:
                    j = k // 2
                    lo = lpool.tile([KT, 2, N], fp8, name="lo", tag="lo")
                    nc.scalar.dma_start(
                        lo[:], a_lo.ap()[k : k + 2].rearrange("i p n -> p i n")
                    )
                    lo_pairs[j] = lo

                if k < NK - 1:
                    # bf16 passes for this k-chunk
                    for pi, se in enumerate((0, 1)):
                        for n in range(NN):
                            nc.tensor.matmul(
                                ps[n][:],
                                e_sb[:, se, k, :],
                                hi[:, n * NT : (n + 1) * NT],
                                start=(k == 0 and pi == 0),
                                stop=False,
                            )
                    if k % 2 == 1:
                        # fp8 DoubleRow pass for the completed pair
                        j = k // 2
                        for n in range(NN):
                            nc.tensor.matmul(
                                ps[n][:],
                                e8_sb[:, j, :, :],
                                lo_pairs[j][:, :, n * NT : (n + 1) * NT],
                                start=False,
                                stop=False,
                                perf_mode=DR,
                            )
                else:
                    # last chunk: bank-major, stores pipelined per bank
                    j = NKH - 1
                    for n in range(NN):
                        for se in (0, 1):
                            nc.tensor.matmul(
                                ps[n][:],
                                e_sb[:, se, k, :],
                                hi[:, n * NT : (n + 1) * NT],
                                start=False,
                                stop=False,
                            )
                        nc.tensor.matmul(
                            ps[n][:],
                            e8_sb[:, j, :, :],
                            lo_pairs[j][:, :, n * NT : (n + 1) * NT],
                            start=False,
                            stop=True,
                            perf_mode=DR,
                        )
                        o_sb = opool.tile(
                            [D, NT], f32, name=f"o{n}", tag=f"o{n}"
                        )
                        nc.vector.tensor_copy(o_sb[:], ps[n][:])
                        (nc.sync if n % 2 == 0 else nc.scalar).dma_start(
                            out_t.ap()[:, n * NT : (n + 1) * NT], o_sb[:]
                        )

    try:
        _dedup_ldweights(nc, mybir)
    except Exception:
        pass
    nc.compile()
    return nc


def _make_in_maps(last_embs, neibors):
    in_maps = []
    for g in range(B):
        at_g = np.ascontiguousarray(neibors[g].T)  # [m, n] f32
        ah = at_g.astype(BF16)
        al = at_g - ah.astype(np.float32)
        al8 = (al * LO_SCALE).astype(FP8)
        eg = np.ascontiguousarray(last_embs[g])
        eh = eg.astype(BF16)
        el = (eg - eh.astype(np.float32)).astype(BF16)
        ehs8 = (eh.astype(np.float32) / LO_SCALE).astype(FP8)  # [N, D]
        e2 = np.stack(
            [eh.reshape(NK, KT, D), el.reshape(NK, KT, D)], axis=0
        ).transpose(0, 2, 1, 3)  # [2, KT, NK, D]
        e8 = ehs8.reshape(NKH, 2, KT, D).transpose(0, 2, 1, 3)  # [NKH,KT,2,D]
        in_maps.append(
            {
                "a_hi": np.ascontiguousarray(ah.reshape(NK, KT, N)),
                "a_lo": np.ascontiguousarray(al8.reshape(NK, KT, N)),
                "e2": np.ascontiguousarray(e2),
                "e8": np.ascontiguousarray(e8),
            }
        )
    return in_maps


def kernel(last_embs, neibors):
    global _cached_nc
    from concourse.bass_utils import run_bass_kernel_spmd

    last_embs = np.asarray(last_embs, dtype=np.float32)
    neibors = np.asarray(neibors, dtype=np.float32)
    if _cached_nc is None:
        _cached_nc = _build_program()
    in_maps = _make_in_maps(last_embs, neibors)
    try:
        res = run_bass_kernel_spmd(_cached_nc, in_maps, list(range(B))).results
    except Exception:
        # transient NRT/terminal hiccups have been observed; retry once
        import time

        time.sleep(15)
        res = run_bass_kernel_spmd(_cached_nc, in_maps, list(range(B))).results
    out = np.stack([res[g]["out_t"].T for g in range(B)], axis=0)
    return np.ascontiguousarray(out).astype(np.float32, copy=False)


# revision 28
# speedup vs baseline: 1.1695x; 1.1695x over previous
"""Batched GNN neighbor aggregation on 8 NeuronCores.

out[b] = neibors[b] @ last_embs[b]  for b in 0..7  (2048x2048 @ 2048x128, f32)

Sharding: one graph per core (batch dim across the 8 cores), no cross-core
communication. The PE contracts over the partition dimension, so the
adjacency operand must sit in SBUF with the contraction index (m) on
partitions; each graph's adjacency is pre-transposed on the host during
sharding so the device streams it with fully-contiguous DMAs.

Precision scheme (TRN2's native fp32 matmul is 4 cycles/row and slower
than the HBM stream): A = bf16 hi + fp8e4m3 lo (lo scaled by 2^9),
E = bf16 hi + bf16 lo. Per k-chunk the device accumulates in f32 PSUM:
  Ah@Eh + Ah@El          (bf16, 1 cycle/row)
  Al8@(Eh * 2^-9 as fp8) (fp8 DoubleRow over k-chunk pairs, 0.5 cyc/row)
The 2^9/2^-9 scales are powers of two and cancel exactly. Measured error
vs the f32 reference: absmax-rel 4.3e-4, resid_var 1.8e-7.

Stream is ~13.8 MB/core (vs 18.4 full-f32), PE ~40 us: measured
~58-62 us per core wall in quiet windows (up to ~70 us when the shared
HBM stacks see external contention), including ~8.5 us fixed NEFF/Tile
preamble and ~6 us tail. A ~3.4us scratch-matmul pre-warm during the DMA
preamble keeps the PE HAM clock at 2.4GHz for the real matmuls. The
device computes out^T = embs^T @ neibors^T with the embedding K-chunks
stationary; the host transposes the small result back.
"""

import numpy as np
import ml_dtypes

BF16 = ml_dtypes.bfloat16
FP8 = ml_dtypes.float8_e4m3
LO_SCALE = np.float32(512.0)

B = 8
N = 2048
D = 128
KT = 128
NT = 512
NK = N // KT   # 16
NKH = NK // 2  # 8 k-chunk pairs for DoubleRow
NN = N // NT   # 4

_cached_nc = None


def _dedup_ldweights(nc, mybir):
    """Drop InstLdweights whose weight AP matches the immediately preceding
    weight load in the PE stream (matmuls here have ldweights=False, so the
    stationary operand stays in the array between identical loads)."""
    for bb in nc.m.functions[0].blocks:
        insts = bb.instructions
        last_key = None
        removed = []
        for inst in insts:
            if getattr(inst, "engine", None) != mybir.EngineType.PE:
                continue
            ty = type(inst).__name__
            if ty == "InstLdweights":
                key = repr(inst.ins[0])
                if key == last_key and not inst.has_wait():
                    removed.append(inst)
                else:
                    last_key = key
            elif ty != "InstMatmult":
                last_key = None
        if removed:
            rm = {id(i) for i in removed}
            insts[:] = [i for i in insts if id(i) not in rm]
            for i in removed:
                nc.inst_map.pop(i.name, None)


def _build_program():
    import concourse.tile as tile
    from concourse import bacc, mybir

    f32 = mybir.dt.float32
    bf16 = mybir.dt.bfloat16
    fp8 = mybir.dt.float8e4
    DR = mybir.MatmulPerfMode.DoubleRow
    nc = bacc.Bacc(
        "TRN2",
        target_bir_lowering=False,
        debug=False,
        enable_asserts=False,
        enable_partition_id=False,
    )

    a_hi = nc.dram_tensor("a_hi", [NK, KT, N], bf16, kind="ExternalInput")
    a_lo = nc.dram_tensor("a_lo", [NK, KT, N], fp8, kind="ExternalInput")
    # e2[plane, p, k, d]: 0 = Eh, 1 = El (bf16)
    e2 = nc.dram_tensor("e2", [2, KT, NK, D], bf16, kind="ExternalInput")
    # e8[j, p, i, d] = fp8(Eh * 2^-9) for k-chunk 2j+i (DoubleRow weights)
    e8 = nc.dram_tensor("e8", [NKH, KT, 2, D], fp8, kind="ExternalInput")
    out_t = nc.dram_tensor("out_t", [D, N], f32, kind="ExternalOutput")

    with tile.TileContext(nc) as tc:
        with (
            tc.tile_pool(name="econst", bufs=1) as epool,
            tc.tile_pool(name="ahi", bufs=12) as hpool,
            tc.tile_pool(name="alo", bufs=6) as lpool,
            tc.tile_pool(name="psum", bufs=1, space="PSUM") as pspool,
            tc.tile_pool(name="out", bufs=1) as opool,
        ):
            # HAM pre-warm: ~3.4us of scratch matmuls during the DMA-wait
            # preamble so the real matmuls start at 2.4GHz, not 1.2GHz.
            wu = epool.tile([KT, KT], bf16, name="wu")
            wu_ps = pspool.tile([KT, KT], f32, name="wups", tag="wups")
            nc.gpsimd.memset(wu[:], 0.0)
            for _ in range(32):
                nc.tensor.matmul(wu_ps[:], wu[:], wu[:], start=True, stop=True)

            e2_r = e2.ap().rearrange("s p k d -> p s k d")
            e_sb = epool.tile([KT, 2, NK, D], bf16)
            e8_sb = epool.tile([KT, NKH, 2, D], fp8, name="e8_sb")
            nc.sync.dma_start(e_sb[:, 0, 0], e2_r[:, 0, 0])
            nc.scalar.dma_start(e_sb[:, 0, 1:], e2_r[:, 0, 1:])
            nc.scalar.dma_start(e_sb[:, 1], e2_r[:, 1])
            nc.scalar.dma_start(e8_sb[:], e8.ap().rearrange("j p i d -> p j i d"))

            ps = [
                pspool.tile([D, NT], f32, name=f"ps{n}", tag=f"ps{n}")
                for n in range(NN)
            ]

            lo_pairs = {}
            for k in range(NK):
                hi = hpool.tile([KT, N], bf16, tag="hi")
                if k == 0:
                    for n in range(NN):
                        nc.sync.dma_start(
                            hi[:, n * NT : (n + 1) * NT],
                            a_hi.ap()[k][:, n * NT : (n + 1) * NT],
                        )
                elif k == NK - 1:
                    for n in range(NN):
                        nc.sync.dma_start(
                            hi[:, n * NT : (n + 1) * NT],
                            a_hi.ap()[k][:, n * NT : (n + 1) * NT],
                        )
                else:
                    nc.sync.dma_start(hi[:], a_hi.ap()[k])
                if k % 2 == 0:
                    j = k // 2
                    lo = lpool.tile([KT, 2, N], fp8, name="lo", tag="lo")
                    nc.scalar.dma_start(
                        lo[:], a_lo.ap()[k : k + 2].rearrange("i p n -> p i n")
                    )
                    lo_pairs[j] = lo

                if k < NK - 1:
                    # bf16 passes for this k-chunk
                    for pi, se in enumerate((0, 1)):
                        for n in range(NN):
                            nc.tensor.matmul(
                                ps[n][:],
                                e_sb[:, se, k, :],
                                hi[:, n * NT : (n + 1) * NT],
                                start=(k == 0 and pi == 0),
                                stop=False,
                            )
                    if k % 2 == 1:
                        # fp8 DoubleRow pass for the completed pair
                        j = k // 2
                        for n in range(NN):
                            nc.tensor.matmul(
                                ps[n][:],
                                e8_sb[:, j, :, :],
                                lo_pairs[j][:, :, n * NT : (n + 1) * NT],
                                start=False,
                                stop=False,
                                perf_mode=DR,
                            )
                else:
                    # last chunk: bank-major, stores pipelined per bank
                    j = NKH - 1
                    for n in range(NN):
                        for se in (0, 1):
                            nc.tensor.matmul(
                                ps[n][:],
                                e_sb[:, se, k, :],
                                hi[:, n * NT : (n + 1) * NT],
                                start=False,
                                stop=False,
                            )
                        nc.tensor.matmul(
                            ps[n][:],
                            e8_sb[:, j, :, :],
                            lo_pairs[j][:, :, n * NT : (n + 1) * NT],
                            start=False,
                            stop=True,
                            perf_mode=DR,
                        )
                        o_sb = opool.tile(
                            [D, NT], f32, name=f"o{n}", tag=f"o{n}"
                        )
                        nc.vector.tensor_copy(o_sb[:], ps[n][:])
                        (nc.sync if n % 2 == 0 else nc.scalar).dma_start(
                            out_t.ap()[:, n * NT : (n + 1) * NT], o_sb[:]
                        )

    try:
        _dedup_ldweights(nc, mybir)
    except Exception:
        pass
    nc.compile()
    return nc


def _make_in_maps(last_embs, neibors):
    in_maps = []
    for g in range(B):
        at_g = np.ascontiguousarray(neibors[g].T)  # [m, n] f32
        ah = at_g.astype(BF16)
        al = at_g - ah.astype(np.float32)
        al8 = (al * LO_SCALE).astype(FP8)
        eg = np.ascontiguousarray(last_embs[g])
        eh = eg.astype(BF16)
        el = (eg - eh.astype(np.float32)).astype(BF16)
        ehs8 = (eh.astype(np.float32) / LO_SCALE).astype(FP8)  # [N, D]
        e2 = np.stack(
            [eh.reshape(NK, KT, D), el.reshape(NK, KT, D)], axis=0
        ).transpose(0, 2, 1, 3)  # [2, KT, NK, D]
        e8 = ehs8.reshape(NKH, 2, KT, D).transpose(0, 2, 1, 3)  # [NKH,KT,2,D]
        in_maps.append(
            {
                "a_hi": np.ascontiguousarray(ah.reshape(NK, KT, N)),
                "a_lo": np.ascontiguousarray(al8.reshape(NK, KT, N)),
                "e2": np.ascontiguousarray(e2),
                "e8": np.ascontiguousarray(e8),
            }
        )
    return in_maps


def kernel(last_embs, neibors):
    global _cached_nc
    from concourse.bass_utils import run_bass_kernel_spmd

    last_embs = np.asarray(last_embs, dtype=np.float32)
    neibors = np.asarray(neibors, dtype=np.float32)
    if _cached_nc is None:
        _cached_nc = _build_program()
    in_maps = _make_in_maps(last_embs, neibors)
    try:
        res = run_bass_kernel_spmd(_cached_nc, in_maps, list(range(B))).results
    except Exception:
        # transient NRT/terminal hiccups have been observed; retry once
        import time

        time.sleep(15)
        res = run_bass_kernel_spmd(_cached_nc, in_maps, list(range(B))).results
    out = np.stack([res[g]["out_t"].T for g in range(B)], axis=0)
    return np.ascontiguousarray(out).astype(np.float32, copy=False)
